# revision 4
# baseline (speedup 1.0000x reference)
"""LogSparseAttention (band |i-j|<=16) Trainium2 Bass kernel.

8 NeuronCores: core c handles batch b=c//2, sequence half c%2 (1024 queries).
Full inputs in, full output out; sharding/padding/masks prepared on host.

Math (per core, all 8 heads):
  qhT/khT = (x @ W)^T computed as W^T-chunk matmuls against PE-transposed
  inputs (fp32 transposes, float32r projections), stored bf16.
  vh = v @ wv in natural layout, stored bf16 in 128-row key tiles with an
  extra all-ones column per head (fused softmax denominator).
  Per q-tile/head: logitsT[key, q] via bf16 matmuls (head pairs packed into
  the PE array via tile_position row groups), exp on ACT (scale=1/8, per-key
  bias folds the bq@khT softmax term), band+validity mask multiply on DVE,
  attn@V as [vh|1]^T @ expT giving yT rows + denominators, reciprocal +
  selector-matmul broadcast, normalize on eviction, then float32r
  out-projection. bq/bk terms that are softmax-invariant are dropped;
  bv@wo+bo added on host.
"""

import sys

sys.path.insert(0, "/opt/trn_rl_repo")

import numpy as np
import ml_dtypes

import bass_rust
import concourse.bass as bass
import concourse.mybir as mybir
import concourse.tile as tile_mod
from concourse.tile import TileContext, ScopedClock
from concourse.masks import make_identity

B, S, D, H = 4, 2048, 512, 8
DEPTH = D // H          # 64
BAND = 16
N_CORES = 8
QR = S // 2             # 1024 queries per core
KR = QR + 2 * BAND      # 1056 k/v rows per core (halo, zero-padded at edges)
NT = QR // 128          # 8 q-tiles per core
SHIFT_T = 9             # key tiles of 128 rows covering KR (9*128=1152)
HS = DEPTH + 1          # 65: vh columns per head incl. ones column
WIN = 128 + 2 * BAND    # 160 window rows per q-tile

f32 = mybir.dt.float32
f32r = mybir.dt.float32r
bf16 = mybir.dt.bfloat16

# ---------------------------------------------------------------- tile patch

_MAXW = 1


def _patched_drain_and_barrier(self, tick_clock, wait_clock):
    nc = self.nc
    carrier = nc.sync.nop()
    wait_clock.add_sem_waits(carrier.ins, ScopedClock({None: tick_clock.global_clock}))
    si = carrier.ins.sync_info
    waits = list(si.on_wait) if si is not None else []
    updates = list(si.on_update) if si is not None else []
    carrier.ins.sync_info = bass_rust.SyncInfo(on_wait=waits[:_MAXW], on_update=updates)
    rest = waits[_MAXW:]
    while rest:
        extra = nc.sync.nop()
        extra.ins.sync_info = bass_rust.SyncInfo(on_wait=rest[:_MAXW], on_update=[])
        rest = rest[_MAXW:]
    nc.sync.drain()
    nc.all_engine_barrier()
    assert self.sems is not None
    popped = nc._tile_sem_poison_stack.pop()
    assert popped is self._sem_poison
    nc.clear_and_free_semaphores(list(self.sems.allocated().values()))
    nc.all_engine_barrier()


tile_mod.TileContext._drain_and_barrier = _patched_drain_and_barrier


def split_multi_waits(nc, max_waits: int = 1):
    """This walrus build accepts at most one sem-wait per instruction; move
    extras onto same-engine NoOps placed immediately before."""
    for fn in nc.m.functions:
        for blk in fn.blocks:
            insts = list(blk.instructions)
            out = []
            changed = False
            for inst in insts:
                si = inst.sync_info
                waits = list(si.on_wait) if si is not None else []
                if len(waits) > max_waits:
                    extras, keep = waits[:-max_waits], waits[-max_waits:]
                    for kk, w in enumerate(extras):
                        nop = bass_rust.InstNoOp(name=f"{inst.name}-sw{kk}", ins=[], outs=[])
                        nop.engine = inst.engine
                        nop.debug = inst.debug
                        nop.sync_info = bass_rust.SyncInfo(on_wait=[w], on_update=[])
                        out.append(nop)
                    inst.sync_info = bass_rust.SyncInfo(
                        on_wait=keep,
                        on_update=list(si.on_update) if si is not None else [],
                    )
                    changed = True
                out.append(inst)
            if changed:
                blk.instructions = out


# ---------------------------------------------------------------- device program


def build_nc():
    nc = bass.Bass()
    qc = nc.declare_dram_parameter("qc", [QR, D], f32, isOutput=False)
    kc = nc.declare_dram_parameter("kc", [KR, D], f32, isOutput=False)
    vc = nc.declare_dram_parameter("vc", [KR, D], f32, isOutput=False)
    wqp = nc.declare_dram_parameter("wq", [D, D], f32, isOutput=False)
    wkp = nc.declare_dram_parameter("wk", [D, D], f32, isOutput=False)
    wvp = nc.declare_dram_parameter("wv", [D, D], f32, isOutput=False)
    wop = nc.declare_dram_parameter("wo", [D, D], f32, isOutput=False)
    maskp = nc.declare_dram_parameter("masks", [NT, WIN, 128], bf16, isOutput=False)
    cbp = nc.declare_dram_parameter("cb", [SHIFT_T, 128, H], f32, isOutput=False)
    outp = nc.declare_dram_parameter("outc", [QR, D], f32, isOutput=True)

    with TileContext(nc) as tc:
        with (
            tc.tile_pool(name="const", bufs=1) as constp,
            tc.tile_pool(name="wpool", bufs=1) as wpool,
            tc.tile_pool(name="persist", bufs=1) as persist,
            tc.tile_pool(name="raw", bufs=2) as rawp,
            tc.tile_pool(name="xt", bufs=2) as xtp,
            tc.tile_pool(name="att", bufs=3) as attp,
            tc.tile_pool(name="yt", bufs=2) as ytp,
            tc.tile_pool(name="osb", bufs=2) as osbp,
            tc.tile_pool(name="pst", bufs=2, space="PSUM") as pst,    # transposes
            tc.tile_pool(name="psbig", bufs=3, space="PSUM") as psbig,  # proj/logits/outproj
            tc.tile_pool(name="psy", bufs=3, space="PSUM") as psy,    # attnV/bcast
        ):
            ident = constp.tile([128, 128], f32)
            make_identity(nc, ident[:])
            ones1 = constp.tile([1, DEPTH], f32, tag="ones1")
            nc.gpsimd.memset(ones1[:], 1.0)

            # weights -> SBUF as float32r via SWDGE cast DMA, [128, 4, 512]
            wsb = {}
            for name, par in (("wq", wqp), ("wk", wkp), ("wv", wvp), ("wo", wop)):
                t = wpool.tile([128, 4, D], f32r, tag=f"w_{name}")
                nc.gpsimd.dma_start(
                    out=t[:], in_=par.rearrange("(kt p) n -> p kt n", p=128)
                )
                wsb[name] = t

            maskA = persist.tile([128, NT, 128], bf16, tag="mA")
            maskB = persist.tile([32, NT, 128], bf16, tag="mB")
            nc.sync.dma_start(
                out=maskA[:], in_=maskp[:, 0:128, :].rearrange("t r q -> r t q")
            )
            nc.sync.dma_start(
                out=maskB[:], in_=maskp[:, 128:WIN, :].rearrange("t r q -> r t q")
            )
            cb_sb = persist.tile([128, SHIFT_T, H], f32, tag="cb")
            nc.sync.dma_start(out=cb_sb[:], in_=cbp.rearrange("t p h -> p t h"))

            qhT = persist.tile([128, 4, QR], bf16, tag="qhT")
            khT = persist.tile([128, 4, KR], bf16, tag="khT")
            vh = persist.tile([128, SHIFT_T, H * HS], bf16, tag="vh")
            nc.gpsimd.memset(vh[:], 1.0)

            # ---------------- phase 1: transposes + projections ----------------
            def do_matrix(par, n_rows, kind):
                offs = []
                o = 0
                while o < n_rows:
                    sz = min(512, n_rows - o)
                    offs.append((o, sz))
                    o += sz
                for (off, sz) in offs:
                    n_st = (sz + 127) // 128
                    p_last = sz - (n_st - 1) * 128
                    psz = min(128, sz)
                    raw = rawp.tile([128, n_st, D], f32, tag="raw")
                    nc.sync.dma_start(
                        out=raw[0:psz, :, :],
                        in_=par[off:off + sz].rearrange("(st p) n -> p st n", p=psz),
                    )
                    xT = xtp.tile([128, 4, 512], f32r, tag="xT")
                    for st in range(n_st):
                        pp = 128 if st < n_st - 1 else p_last
                        for dc in range(4):
                            pt = pst.tile([128, 128], f32, tag="pt")
                            nc.tensor.transpose(
                                pt[:, 0:pp],
                                raw[0:pp, st, dc * 128:(dc + 1) * 128],
                                ident[0:pp, 0:pp],
                            )
                            nc.vector.tensor_copy(
                                xT[:, dc, st * 128:st * 128 + pp], pt[:, 0:pp]
                            )
                    if kind in ("q", "k"):
                        w = wsb["wq"] if kind == "q" else wsb["wk"]
                        dst = qhT if kind == "q" else khT
                        for m in range(4):
                            pb = psbig.tile([128, 512], f32, tag="big")
                            for kt in range(4):
                                nc.tensor.matmul(
                                    pb[:, 0:sz],
                                    w[:, kt, m * 128:(m + 1) * 128],
                                    xT[:, kt, 0:sz],
                                    start=(kt == 0),
                                    stop=(kt == 3),
                                )
                            nc.scalar.copy(dst[:, m, off:off + sz], pb[:, 0:sz])
                    else:  # v: natural layout, 128-row key tiles, 65-stride heads
                        for st in range(n_st):
                            pp = 128 if st < n_st - 1 else p_last
                            pv = psbig.tile([128, 512], f32, tag="big")
                            for kt in range(4):
                                nc.tensor.matmul(
                                    pv[0:pp, :],
                                    xT[:, kt, st * 128:st * 128 + pp],
                                    wsb["wv"][:, kt, :],
                                    start=(kt == 0),
                                    stop=(kt == 3),
                                )
                            T = (off + st * 128) // 128
                            dstv = vh[0:pp, T, :].rearrange("p (h e) -> p h e", h=H)
                            nc.vector.tensor_copy(
                                dstv[:, :, 0:DEPTH],
                                pv[0:pp, :].rearrange("p (h e) -> p h e", h=H),
                            )

            do_matrix(qc, QR, "q")
            do_matrix(kc, KR, "k")
            do_matrix(vc, KR, "v")

            # ---------------- phase 2: attention + out-projection ----------------
            for t in range(NT):
                yT = ytp.tile([128, 4, 128], f32r, tag="yT")
                for p in range(4):
                    for hh in range(2):
                        h = 2 * p + hh
                        po = 64 * hh
                        colA = t * 128  # khT col of first window row (key t*128-16)
                        pA = psbig.tile([128, 512], f32, tag="big")
                        nc.tensor.matmul(
                            pA[:, 0:128],
                            khT[po:po + 64, p, colA:colA + 128],
                            qhT[po:po + 64, p, t * 128:(t + 1) * 128],
                            start=True, stop=True,
                            tile_position=(po, 0),
                        )
                        pB = psbig.tile([32, 512], f32, tag="big")
                        nc.tensor.matmul(
                            pB[:, 0:128],
                            khT[po:po + 64, p, colA + 128:colA + 160],
                            qhT[po:po + 64, p, t * 128:(t + 1) * 128],
                            start=True, stop=True,
                            tile_position=(po, 0),
                        )
                        eA = attp.tile([128, 128], bf16, tag="eA")
                        eB = attp.tile([32, 128], bf16, tag="eB")
                        nc.scalar.activation(
                            eA[:], pA[:, 0:128], mybir.ActivationFunctionType.Exp,
                            bias=cb_sb[:, t, h:h + 1], scale=0.125,
                        )
                        nc.scalar.activation(
                            eB[:], pB[:, 0:128], mybir.ActivationFunctionType.Exp,
                            bias=cb_sb[0:32, t + 1, h:h + 1], scale=0.125,
                        )
                        nc.vector.tensor_mul(eA[:], eA[:], maskA[:, t, :])
                        nc.vector.tensor_mul(eB[:], eB[:], maskB[:, t, :])
                        pY = psy.tile([HS, 128], f32, tag="py")
                        nc.tensor.matmul(
                            pY[:], vh[:, t, h * HS:(h + 1) * HS], eA[:],
                            start=True, stop=False,
                        )
                        nc.tensor.matmul(
                            pY[:], vh[0:32, t + 1, h * HS:(h + 1) * HS], eB[:],
                            start=False, stop=True,
                        )
                        rr = attp.tile([1, 128], f32, tag="rr")
                        nc.vector.reciprocal(rr[:], pY[DEPTH:HS, :])
                        pBC = psy.tile([DEPTH, 128], f32, tag="py")
                        nc.tensor.matmul(pBC[:], ones1[:], rr[:], start=True, stop=True)
                        bc_sb = attp.tile([DEPTH, 128], f32, tag="bc")
                        nc.scalar.copy(bc_sb[:], pBC[:])
                        nc.vector.tensor_mul(
                            yT[64 * hh:64 * hh + 64, p, :],
                            pY[0:DEPTH, :],
                            bc_sb[:],
                        )
                pO = psbig.tile([128, 512], f32, tag="big")
                for m in range(4):
                    nc.tensor.matmul(
                        pO[:], yT[:, m, :], wsb["wo"][:, m, :],
                        start=(m == 0), stop=(m == 3),
                    )
                ot = osbp.tile([128, D], f32, tag="ot")
                nc.scalar.copy(ot[:], pO[:])
                nc.sync.dma_start(out=outp[t * 128:(t + 1) * 128, :], in_=ot[:])

    split_multi_waits(nc)
    return nc


# ---------------------------------------------------------------- host wrapper

_CACHE = {}


def _host_inputs(q, k, v, wq, bq, wk, bk, wv, bv, wo, bo):
    in_maps = []
    wq32 = np.ascontiguousarray(wq, np.float32)
    wk32 = np.ascontiguousarray(wk, np.float32)
    wv32 = np.ascontiguousarray(wv, np.float32)
    wo32 = np.ascontiguousarray(wo, np.float32)
    for c in range(N_CORES):
        b, half = divmod(c, 2)
        q0 = half * QR
        qcore = np.ascontiguousarray(q[b, q0:q0 + QR]).astype(np.float32)
        lo, hi = q0 - BAND, q0 + QR + BAND
        kcz = np.zeros((KR, D), np.float32)
        vcz = np.zeros((KR, D), np.float32)
        s0, s1 = max(0, lo), min(S, hi)
        kcz[s0 - lo:s1 - lo] = k[b, s0:s1]
        vcz[s0 - lo:s1 - lo] = v[b, s0:s1]

        masks = np.zeros((NT, WIN, 128), np.float32)
        r = np.arange(WIN)[:, None]
        i = np.arange(128)[None, :]
        for t in range(NT):
            kg = q0 + 128 * t - BAND + r          # global key index
            band = (i <= r) & (r <= i + 2 * BAND)
            valid = (kg >= 0) & (kg < S)
            masks[t] = (band & valid).astype(np.float32)
        masks = masks.astype(ml_dtypes.bfloat16)

        cb = np.zeros((SHIFT_T, 128, H), np.float32)
        if np.any(bq):
            khc = kcz @ wk32  # bk term is softmax-invariant, dropped
            cvals = np.einsum(
                "jhe,he->jh", khc.reshape(KR, H, DEPTH),
                np.asarray(bq, np.float32).reshape(H, DEPTH),
            ) / 8.0
            flat = np.zeros((SHIFT_T * 128, H), np.float32)
            flat[:KR] = cvals
            cb = flat.reshape(SHIFT_T, 128, H)

        in_maps.append({
            "qc": qcore, "kc": kcz, "vc": vcz,
            "wq": wq32, "wk": wk32, "wv": wv32, "wo": wo32,
            "masks": masks, "cb": cb,
        })
    return in_maps


def kernel(q, k, v, wq, bq, wk, bk, wv, bv, wo, bo):
    from concourse.bass_utils import run_bass_kernel_spmd

    q = np.asarray(q); k = np.asarray(k); v = np.asarray(v)
    wq = np.asarray(wq); wk = np.asarray(wk); wv = np.asarray(wv)
    wo = np.asarray(wo)
    bq = np.asarray(bq); bk = np.asarray(bk); bv = np.asarray(bv)
    bo = np.asarray(bo)

    if "nc" not in _CACHE:
        _CACHE["nc"] = build_nc()
    nc = _CACHE["nc"]
    in_maps = _host_inputs(q, k, v, wq, bq, wk, bk, wv, bv, wo, bo)

    last_err = None
    res = None
    for _attempt in range(3):
        try:
            res = run_bass_kernel_spmd(nc, in_maps, core_ids=list(range(N_CORES)))
            break
        except Exception as e:  # flaky axon device errors
            last_err = e
    if res is None:
        raise last_err

    out = np.empty((B, S, D), np.float32)
    for c in range(N_CORES):
        b, half = divmod(c, 2)
        out[b, half * QR:(half + 1) * QR] = res.results[c]["outc"]
    out += (np.asarray(bv, np.float32) @ wo + np.asarray(bo, np.float32))[None, None, :]
    return out.astype(np.float32)
